# revision 8
# baseline (speedup 1.0000x reference)
"""LOCA-style kernel for Trainium2, data-parallel over batch on 8 NeuronCores.

Per core (one batch element), per step:
  - depthwise 3x3 correlation computed as three convs D0=conv(w0-w2),
    D1=conv(w1-w2), R2=conv(w2) via fp8e4 DoubleRow matmuls (2 taps per
    matmul) on a flat-raster fp8 feature map with zero-edge-column
    variants; center-tap weight quantization error is corrected through
    the spare DoubleRow k-tile.
  - softmax-weighted object sum via the shift identity
      red = R2 + (D0*e0 + D1*e1) / (1 + e0 + e1),  e_i = exp(D_i)
    exps evacuate PSUM on ScalarE; products read PSUM on DVE; 1/den is
    exp(-ln(.)) with |w_head| folded into the ln scale/bias.
  - 1x1 head with sign(w_head) stationary + ReLU + 8x bilinear upsample
    as two separable matmul passes. Output fp16, upcast on host.
"""

import sys

sys.path.insert(0, "/opt/trn_rl_repo")

import numpy as np
from contextlib import ExitStack

import concourse.bass as bass
import concourse.mybir as mybir
from concourse import bacc, tile
from concourse.ap import AP
from concourse.bass_utils import run_bass_kernel_spmd

BS, C, H, W = 8, 256, 64, 64
STEPS, NO = 3, 3
RED = 8
HO, WO = H * RED, W * RED  # 512, 512
NCORES = 8
NCT = 2
HW = H * W  # 4096
GR = 1024  # psum granule (pixels)
NGR = HW // GR  # 4
F16 = mybir.dt.float16
F32 = mybir.dt.float32
F8 = mybir.dt.float8e4
AF = mybir.ActivationFunctionType
ALU = mybir.AluOpType
PM = mybir.MatmulPerfMode

# flat f8 layout: one tile [128, 3*BLK] per ct;
#   block 0 (Vz63): col x=63 zeroed, x-origin at 65 (for dx=-1 taps)
#   block 1 (Vz0):  col x=0 zeroed, x-origin at 65 (for dx=+1 taps)
#   block 2 (V0):   full map, x-origin at 64 (for dx=0 taps)
BLK = 4232  # 64 head pad + 4096 + 72 tail pad, even
B1, B2, B3 = 0, BLK, 2 * BLK
FTOT = 3 * BLK


def _tap_off(dy, dx):
    if dx == -1:
        return B1 + 64 + 64 * dy
    if dx == 1:
        return B2 + 66 + 64 * dy
    return B3 + 64 + 64 * dy


# PE taps (x-shifted + center, fp16 diag matmuls); DVE taps: (-1,0),(+1,0)
PTAPS = [(-1, -1), (-1, 1), (0, -1), (0, 0), (0, 1), (1, -1), (1, 1)]
NPT = len(PTAPS)
PTAP_OFFS = [_tap_off(*t) for t in PTAPS]
for _o in PTAP_OFFS:
    assert _o % 2 == 0, _o
DYTAP_OFFS = [_tap_off(-1, 0), _tap_off(1, 0)]


def _bilinear_matrix(n_in: int, n_out: int) -> np.ndarray:
    U = np.zeros((n_out, n_in), np.float64)
    s = n_in / n_out
    for i in range(n_out):
        c = (i + 0.5) * s - 0.5
        lo = int(np.floor(c))
        f = c - lo
        for idx, wt in ((lo, 1.0 - f), (lo + 1, f)):
            U[i, min(max(idx, 0), n_in - 1)] += wt
    return U


def _host_prep(f_e, all_prototypes, w_head, b_head):
    import ml_dtypes

    E4 = ml_dtypes.float8_e4m3
    f_e = np.asarray(f_e, np.float32)
    ap = np.asarray(all_prototypes, np.float32)
    w_head = np.asarray(w_head, np.float32)
    b_val = float(np.asarray(b_head).reshape(-1)[0])

    # ---- flat fp16 variants ----
    f16 = f_e.astype(np.float16)  # [BS, C, H, W]
    z63 = f16.copy()
    z63[:, :, :, 63] = 0
    z0 = f16.copy()
    z0[:, :, :, 0] = 0
    fblk = np.zeros((BS, NCT, 128, FTOT), np.float16)
    for ct in range(NCT):
        sl = slice(ct * 128, (ct + 1) * 128)
        fblk[:, ct, :, B1 + 65: B1 + 65 + HW] = z63[:, sl].reshape(BS, 128, HW)
        fblk[:, ct, :, B2 + 65: B2 + 65 + HW] = z0[:, sl].reshape(BS, 128, HW)
        fblk[:, ct, :, B3 + 64: B3 + 64 + HW] = f16[:, sl].reshape(BS, 128, HW)

    # ---- conv weights: D0 = w0-w2, D1 = w1-w2, R2 = w2 (fp16 exact) ----
    # ap[s, o*9+t, b, c] -> v[b, s, conv, t, c]
    wm = ap.transpose(2, 0, 1, 3).reshape(BS, STEPS, NO, 9, C)
    v = np.stack([wm[:, :, 0] - wm[:, :, 2], wm[:, :, 1] - wm[:, :, 2], wm[:, :, 2]], axis=2)
    vf = v.astype(np.float16).astype(np.float32)  # [BS, S, 3, 9, C]

    # PE-tap diag stationaries [BS, S, NCT, 3conv, 7tap, 128, 128] fp16
    diag = np.zeros((BS, STEPS, NCT, 3, NPT, 128, 128), np.float16)
    cidx = np.arange(128)
    tapidx = lambda dy, dx: (dy + 1) * 3 + (dx + 1)
    for ct in range(NCT):
        sl = slice(ct * 128, (ct + 1) * 128)
        for pi, tp in enumerate(PTAPS):
            wa = vf[:, :, :, tapidx(*tp), sl]  # [BS, S, 3, 128]
            diag[:, :, ct, :, pi, cidx, cidx] = wa.transpose(3, 0, 1, 2).astype(np.float16)
    # reorder for contiguous per-step DMA: -> [BS, S, 128, NCT, 3, 7, 128]
    diag = np.ascontiguousarray(diag.transpose(0, 1, 5, 2, 3, 4, 6))

    # DVE tap scalars (dy=-1,+1; dx=0): [BS, S, NCT*3*2, 128... store [BS, S, 128, NCT*3*2] f32
    wdy = np.zeros((BS, STEPS, 128, NCT * 3 * 2), np.float32)
    for ct in range(NCT):
        sl = slice(ct * 128, (ct + 1) * 128)
        for cv in range(3):
            for i, dy in enumerate((-1, 1)):
                wdy[:, :, :, (ct * 3 + cv) * 2 + i] = vf[:, :, cv, tapidx(dy, 0), sl]

    absw = np.abs(w_head).astype(np.float64)
    invw = np.where(absw > 0, 1.0 / np.maximum(absw, 1e-30), 1.0e30)
    invw = np.minimum(invw, 1.0e30).astype(np.float32)
    absw_f = absw.astype(np.float32)
    signw = np.sign(w_head).astype(np.float16)
    invw_t = np.ascontiguousarray(invw.reshape(NCT, 128, 1))
    absw_t = np.ascontiguousarray(absw_f.reshape(NCT, 128, 1))
    signw_t = np.ascontiguousarray(signw.reshape(NCT, 128, 1))

    ut = _bilinear_matrix(H, HO).T.astype(np.float16)  # [64, 512]
    eye = np.eye(128, dtype=np.float16)

    in_maps = []
    for b in range(BS):
        in_maps.append(
            {
                "fblk": np.ascontiguousarray(fblk[b]),
                "diag": np.ascontiguousarray(diag[b]),
                "wdy": np.ascontiguousarray(wdy[b]),
                "invw": invw_t,
                "absw": absw_t,
                "signw": signw_t,
                "ut": ut,
                "eye": eye,
            }
        )
    return in_maps, b_val


def _build_nc(b_val: float) -> bass.Bass:
    nc = bacc.Bacc(None, target_bir_lowering=False)
    fblk_d = nc.declare_dram_parameter("fblk", [NCT, 128, FTOT], F16, isOutput=False)
    diag_d = nc.declare_dram_parameter("diag", [STEPS, 128, NCT * 3 * NPT * 128], F16, isOutput=False)
    wdy_d = nc.declare_dram_parameter("wdy", [STEPS, 128, NCT * 3 * 2], F32, isOutput=False)
    invw_d = nc.declare_dram_parameter("invw", [NCT, 128, 1], F32, isOutput=False)
    absw_d = nc.declare_dram_parameter("absw", [NCT, 128, 1], F32, isOutput=False)
    signw_d = nc.declare_dram_parameter("signw", [NCT, 128, 1], F16, isOutput=False)
    ut_d = nc.declare_dram_parameter("ut", [64, WO], F16, isOutput=False)
    eye_d = nc.declare_dram_parameter("eye", [128, 128], F16, isOutput=False)
    out_d = nc.declare_dram_parameter("out", [STEPS, HO, WO], F16, isOutput=True)

    with tile.TileContext(nc) as tc, ExitStack() as ctx:
        const = ctx.enter_context(tc.tile_pool(name="const", bufs=1))
        fpool = ctx.enter_context(tc.tile_pool(name="fpool", bufs=1))
        dpool = ctx.enter_context(tc.tile_pool(name="dpool", bufs=1))
        upool = ctx.enter_context(tc.tile_pool(name="upool", bufs=2))
        vpool = ctx.enter_context(tc.tile_pool(name="vpool", bufs=1))
        tpool = ctx.enter_context(tc.tile_pool(name="tpool", bufs=2))
        rpool = ctx.enter_context(tc.tile_pool(name="rpool", bufs=2))
        opool = ctx.enter_context(tc.tile_pool(name="opool", bufs=2))
        ps_conv = ctx.enter_context(tc.tile_pool(name="ps_conv", bufs=2, space="PSUM"))
        ps_head = ctx.enter_context(tc.tile_pool(name="ps_head", bufs=1, space="PSUM"))

        # ---- constants ----
        ut_sb = const.tile([64, WO], F16, tag="ut")
        nc.sync.dma_start(out=ut_sb[:], in_=ut_d[:])
        eye_sb = const.tile([128, 128], F16, tag="eye")
        nc.sync.dma_start(out=eye_sb[:], in_=eye_d[:])
        invw_sb, absw_sb, signw_sb = [], [], []
        for ct in range(NCT):
            t = const.tile([128, 1], F32, tag=f"invw{ct}")
            nc.sync.dma_start(out=t[:], in_=invw_d[ct])
            invw_sb.append(t)
            t = const.tile([128, 1], F32, tag=f"absw{ct}")
            nc.sync.dma_start(out=t[:], in_=absw_d[ct])
            absw_sb.append(t)
            t = const.tile([128, 1], F16, tag=f"signw{ct}")
            nc.sync.dma_start(out=t[:], in_=signw_d[ct])
            signw_sb.append(t)

        # ---- fp16 flat variants ----
        fsb = []
        for ct in range(NCT):
            t = fpool.tile([128, FTOT], F16, tag=f"f{ct}")
            nc.sync.dma_start(out=t[:], in_=fblk_d[ct])
            fsb.append(t)

        for s in range(STEPS):
            # stationaries for this step
            dg = dpool.tile([128, NCT * 3 * NPT * 128], F16, tag="diag")
            nc.sync.dma_start(out=dg[:], in_=diag_d[s])
            dgv = dg[:].rearrange("p (ct c pt x) -> p ct c pt x", ct=NCT, c=3, pt=NPT)
            wy = dpool.tile([128, NCT * 3 * 2], F32, tag="wdy")
            nc.sync.dma_start(out=wy[:], in_=wdy_d[s])

            redw = rpool.tile([128, NCT * HW], F16, tag="redw")
            for ct in range(NCT):
                e0 = upool.tile([128, HW], F16, tag="e0")
                e1 = upool.tile([128, HW], F16, tag="e1")
                resp = []
                for cv in range(3):
                    rsp = vpool.tile([128, HW], F16, tag=f"rsp{cv}")
                    resp.append(rsp)
                    tmp0 = tpool.tile([128, HW], F16, tag="tmp0")
                    tmp1 = tpool.tile([128, HW], F16, tag="tmp1")
                    wix = (ct * 3 + cv) * 2
                    nc.vector.tensor_scalar(
                        tmp0[:], fsb[ct][:, DYTAP_OFFS[0]: DYTAP_OFFS[0] + HW],
                        wy[:, wix: wix + 1], None, op0=ALU.mult)
                    nc.vector.tensor_scalar(
                        tmp1[:], fsb[ct][:, DYTAP_OFFS[1]: DYTAP_OFFS[1] + HW],
                        wy[:, wix + 1: wix + 2], None, op0=ALU.mult)
                    for g in range(NGR):
                        acc = ps_conv.tile([128, GR], F32, tag="acc")
                        for pi in range(NPT):
                            stat = dgv[:, ct, cv, pi, :]
                            off = PTAP_OFFS[pi] + g * GR
                            for sub in range(GR // 512):
                                nc.tensor.matmul(
                                    acc[:, sub * 512: (sub + 1) * 512],
                                    stat,
                                    fsb[ct][:, off + sub * 512: off + (sub + 1) * 512],
                                    start=(pi == 0),
                                    stop=(pi == NPT - 1),
                                )
                        gsl = slice(g * GR, (g + 1) * GR)
                        nc.vector.tensor_add(rsp[:, gsl], acc[:], tmp0[:, gsl])
                    nc.vector.tensor_add(rsp[:], rsp[:], tmp1[:])

                D0r, D1r, R2r = resp
                # e = exp(D) on ScalarE
                nc.scalar.activation(e0[:], D0r[:], AF.Exp)
                nc.scalar.activation(e1[:], D1r[:], AF.Exp)
                # t = D*e in-place; u = t0+t1 into D0r
                nc.vector.tensor_mul(D0r[:], D0r[:], e0[:])
                nc.vector.tensor_mul(D1r[:], D1r[:], e1[:])
                nc.vector.tensor_add(D0r[:], D0r[:], D1r[:])
                # s01 = e0 + e1 on GPSIMD (in-place e0)
                nc.gpsimd.tensor_add(e0[:], e0[:], e1[:])
                # lnd = Ln((s01+1)*invw); rw = exp(-lnd) = |w|/den, into e1
                nc.scalar.activation(e1[:], e0[:], AF.Ln, scale=invw_sb[ct][:, 0:1], bias=invw_sb[ct][:, 0:1])
                nc.scalar.activation(e1[:], e1[:], AF.Exp, scale=-1.0)
                # v = u * rw (into D0r)
                nc.vector.tensor_mul(D0r[:], D0r[:], e1[:])
                # r2w = |w| * R2 (into R2r)
                nc.vector.tensor_scalar(R2r[:], R2r[:], absw_sb[ct][:, 0:1], None, op0=ALU.mult)
                # redw_half = v + r2w on GPSIMD
                nc.gpsimd.tensor_add(redw[:, ct * HW: (ct + 1) * HW], D0r[:], R2r[:])

            # ---- head: dmap[1, pix] = sum_c sign(w)*redw ----
            dmY = opool.tile([64, 64], F16, tag="dmY")
            for k in range(HW // 512):
                pd = ps_head.tile([1, 512], F32, tag="pd")
                for ct in range(NCT):
                    nc.tensor.matmul(
                        pd[:],
                        signw_sb[ct][:],
                        redw[:, ct * HW + k * 512: ct * HW + (k + 1) * 512],
                        start=(ct == 0),
                        stop=(ct == NCT - 1),
                    )
                dm1 = opool.tile([1, 512], F16, tag="dm1")
                nc.vector.tensor_scalar(dm1[:], pd[:], b_val, 0.0, op0=ALU.add, op1=ALU.max)
                nc.sync.dma_start(
                    out=dmY[8 * k: 8 * k + 8, :],
                    in_=dm1[:].rearrange("p (y x) -> p y x", x=64),
                )
            # transpose -> x on partitions
            psT0 = ps_head.tile([64, 64], F16, tag="ups")
            nc.tensor.transpose(psT0[:], dmY[:], eye_sb[0:64, 0:64])
            dmX = opool.tile([64, 64], F16, tag="dmX")
            nc.vector.tensor_copy(dmX[:], psT0[:])

            # horizontal upsample
            ps_h = ps_head.tile([128, 512], F32, tag="ups")
            for xc in range(4):
                nc.tensor.matmul(
                    ps_h[:, xc * 64: (xc + 1) * 64],
                    ut_sb[:, xc * 128: (xc + 1) * 128],
                    dmX[:],
                    start=True,
                    stop=True,
                )
            h_sb = opool.tile([128, 256], F16, tag="h_sb")
            nc.scalar.activation(h_sb[:], ps_h[:, 0:256], AF.Copy)
            hyT = opool.tile([64, 512], F16, tag="hyT")
            for xc in range(4):
                psTx = ps_head.tile([64, 128], F16, tag="ups")
                nc.tensor.transpose(psTx[:], h_sb[:, xc * 64: (xc + 1) * 64], eye_sb[:])
                nc.vector.tensor_copy(hyT[:, xc * 128: (xc + 1) * 128], psTx[:])
            # vertical upsample
            for yc in range(4):
                pv = ps_head.tile([128, 512], F32, tag="ups")
                nc.tensor.matmul(
                    pv[:],
                    ut_sb[:, yc * 128: (yc + 1) * 128],
                    hyT[:],
                    start=True,
                    stop=True,
                )
                osb = opool.tile([128, 512], F16, tag="osb")
                nc.vector.tensor_copy(osb[:], pv[:])
                nc.sync.dma_start(out=out_d[s, yc * 128: (yc + 1) * 128, :], in_=osb[:])

    nc.compile()
    return nc


_CACHE = {}


def _get_nc(b_val: float) -> bass.Bass:
    key = round(b_val, 12)
    if key not in _CACHE:
        _CACHE[key] = _build_nc(b_val)
    return _CACHE[key]


def kernel(f_e, all_prototypes, w_head, b_head):
    in_maps, b_val = _host_prep(f_e, all_prototypes, w_head, b_head)
    nc = _get_nc(b_val)
    res = run_bass_kernel_spmd(nc, in_maps, list(range(NCORES)), trace=False)
    outs = [res.results[b]["out"].reshape(STEPS, 1, HO, WO) for b in range(BS)]
    full = np.stack(outs, axis=1)  # [STEPS, BS, 1, HO, WO]
    return full.astype(np.float32)


# revision 10
# speedup vs baseline: 1.2323x; 1.2323x over previous
"""LOCA-style kernel for Trainium2, data-parallel over batch on 8 NeuronCores.

Per core (one batch element), per step:
  - depthwise 3x3 correlation computed as three convs D0=conv(w0-w2),
    D1=conv(w1-w2), R2=conv(w2) via fp8e4 DoubleRow matmuls (2 taps per
    matmul) on a flat-raster fp8 feature map with zero-edge-column
    variants; center-tap weight quantization error is corrected through
    the spare DoubleRow k-tile.
  - softmax-weighted object sum via the shift identity
      red = R2 + (D0*e0 + D1*e1) / (1 + e0 + e1),  e_i = exp(D_i)
    exps evacuate PSUM on ScalarE; products read PSUM on DVE; 1/den is
    exp(-ln(.)) with |w_head| folded into the ln scale/bias.
  - 1x1 head with sign(w_head) stationary + ReLU + 8x bilinear upsample
    as two separable matmul passes. Output fp16, upcast on host.
"""

import sys

sys.path.insert(0, "/opt/trn_rl_repo")

import numpy as np
from contextlib import ExitStack

import concourse.bass as bass
import concourse.mybir as mybir
from concourse import bacc, tile
from concourse.ap import AP
from concourse.bass_utils import run_bass_kernel_spmd

BS, C, H, W = 8, 256, 64, 64
STEPS, NO = 3, 3
RED = 8
HO, WO = H * RED, W * RED  # 512, 512
NCORES = 8
NCT = 2
HW = H * W  # 4096
GR = 1024  # psum granule (pixels)
NGR = HW // GR  # 4
F16 = mybir.dt.float16
F32 = mybir.dt.float32
F8 = mybir.dt.float8e4
AF = mybir.ActivationFunctionType
ALU = mybir.AluOpType
PM = mybir.MatmulPerfMode

# flat f8 layout: one tile [128, 3*BLK] per ct;
#   block 0 (Vz63): col x=63 zeroed, x-origin at 65 (for dx=-1 taps)
#   block 1 (Vz0):  col x=0 zeroed, x-origin at 65 (for dx=+1 taps)
#   block 2 (V0):   full map, x-origin at 64 (for dx=0 taps)
BLK = 4232  # 64 head pad + 4096 + 72 tail pad, even
B1, B2, B3 = 0, BLK, 2 * BLK
FTOT = 3 * BLK


def _tap_off(dy, dx):
    if dx == -1:
        return B1 + 64 + 64 * dy
    if dx == 1:
        return B2 + 66 + 64 * dy
    return B3 + 64 + 64 * dy


# PE taps (x-shifted + center, fp16 diag matmuls); DVE taps: (-1,0),(+1,0)
PTAPS = [(-1, -1), (-1, 1), (0, -1), (0, 0), (0, 1), (1, -1), (1, 1)]
NPT = len(PTAPS)
PTAP_OFFS = [_tap_off(*t) for t in PTAPS]
for _o in PTAP_OFFS:
    assert _o % 2 == 0, _o
DYTAP_OFFS = [_tap_off(-1, 0), _tap_off(1, 0)]


def _bilinear_matrix(n_in: int, n_out: int) -> np.ndarray:
    U = np.zeros((n_out, n_in), np.float64)
    s = n_in / n_out
    for i in range(n_out):
        c = (i + 0.5) * s - 0.5
        lo = int(np.floor(c))
        f = c - lo
        for idx, wt in ((lo, 1.0 - f), (lo + 1, f)):
            U[i, min(max(idx, 0), n_in - 1)] += wt
    return U


def _host_prep(f_e, all_prototypes, w_head, b_head):
    import ml_dtypes

    E4 = ml_dtypes.float8_e4m3
    f_e = np.asarray(f_e, np.float32)
    ap = np.asarray(all_prototypes, np.float32)
    w_head = np.asarray(w_head, np.float32)
    b_val = float(np.asarray(b_head).reshape(-1)[0])

    # ---- flat fp16 variants ----
    f16 = f_e.astype(np.float16)  # [BS, C, H, W]
    z63 = f16.copy()
    z63[:, :, :, 63] = 0
    z0 = f16.copy()
    z0[:, :, :, 0] = 0
    fblk = np.zeros((BS, NCT, 128, FTOT), np.float16)
    for ct in range(NCT):
        sl = slice(ct * 128, (ct + 1) * 128)
        fblk[:, ct, :, B1 + 65: B1 + 65 + HW] = z63[:, sl].reshape(BS, 128, HW)
        fblk[:, ct, :, B2 + 65: B2 + 65 + HW] = z0[:, sl].reshape(BS, 128, HW)
        fblk[:, ct, :, B3 + 64: B3 + 64 + HW] = f16[:, sl].reshape(BS, 128, HW)

    # ---- conv weights: D0 = w0-w2, D1 = w1-w2, R2 = w2 (fp16 exact) ----
    # ap[s, o*9+t, b, c] -> v[b, s, conv, t, c]
    wm = ap.transpose(2, 0, 1, 3).reshape(BS, STEPS, NO, 9, C)
    v = np.stack([wm[:, :, 0] - wm[:, :, 2], wm[:, :, 1] - wm[:, :, 2], wm[:, :, 2]], axis=2)
    vf = v.astype(np.float16).astype(np.float32)  # [BS, S, 3, 9, C]

    # PE-tap diag stationaries [BS, S, NCT, 3conv, 7tap, 128, 128] fp16
    diag = np.zeros((BS, STEPS, NCT, 3, NPT, 128, 128), np.float16)
    cidx = np.arange(128)
    tapidx = lambda dy, dx: (dy + 1) * 3 + (dx + 1)
    for ct in range(NCT):
        sl = slice(ct * 128, (ct + 1) * 128)
        for pi, tp in enumerate(PTAPS):
            wa = vf[:, :, :, tapidx(*tp), sl]  # [BS, S, 3, 128]
            diag[:, :, ct, :, pi, cidx, cidx] = wa.transpose(3, 0, 1, 2).astype(np.float16)
    # reorder for contiguous per-step DMA: -> [BS, S, 128, NCT, 3, 7, 128]
    diag = np.ascontiguousarray(diag.transpose(0, 1, 5, 2, 3, 4, 6))

    # DVE tap scalars (dy=-1,+1; dx=0): [BS, S, NCT*3*2, 128... store [BS, S, 128, NCT*3*2] f32
    wdy = np.zeros((BS, STEPS, 128, NCT * 3 * 2), np.float32)
    for ct in range(NCT):
        sl = slice(ct * 128, (ct + 1) * 128)
        for cv in range(3):
            for i, dy in enumerate((-1, 1)):
                wdy[:, :, :, (ct * 3 + cv) * 2 + i] = vf[:, :, cv, tapidx(dy, 0), sl]

    absw = np.abs(w_head).astype(np.float64)
    invw = np.where(absw > 0, 1.0 / np.maximum(absw, 1e-30), 1.0e30)
    invw = np.minimum(invw, 1.0e30).astype(np.float32)
    absw_f = absw.astype(np.float32)
    signw = np.sign(w_head).astype(np.float16)
    invw_t = np.ascontiguousarray(invw.reshape(NCT, 128, 1))
    absw_t = np.ascontiguousarray(absw_f.reshape(NCT, 128, 1))
    signw_t = np.ascontiguousarray(signw.reshape(NCT, 128, 1))

    ut = _bilinear_matrix(H, HO).T.astype(np.float16)  # [64, 512]
    eye = np.eye(128, dtype=np.float16)

    in_maps = []
    for b in range(BS):
        in_maps.append(
            {
                "fblk": np.ascontiguousarray(fblk[b]),
                "diag": np.ascontiguousarray(diag[b]),
                "wdy": np.ascontiguousarray(wdy[b]),
                "invw": invw_t,
                "absw": absw_t,
                "signw": signw_t,
                "ut": ut,
                "eye": eye,
            }
        )
    return in_maps, b_val


def _build_nc(b_val: float) -> bass.Bass:
    nc = bacc.Bacc(None, target_bir_lowering=False)
    fblk_d = nc.declare_dram_parameter("fblk", [NCT, 128, FTOT], F16, isOutput=False)
    diag_d = nc.declare_dram_parameter("diag", [STEPS, 128, NCT * 3 * NPT * 128], F16, isOutput=False)
    wdy_d = nc.declare_dram_parameter("wdy", [STEPS, 128, NCT * 3 * 2], F32, isOutput=False)
    invw_d = nc.declare_dram_parameter("invw", [NCT, 128, 1], F32, isOutput=False)
    absw_d = nc.declare_dram_parameter("absw", [NCT, 128, 1], F32, isOutput=False)
    signw_d = nc.declare_dram_parameter("signw", [NCT, 128, 1], F16, isOutput=False)
    ut_d = nc.declare_dram_parameter("ut", [64, WO], F16, isOutput=False)
    eye_d = nc.declare_dram_parameter("eye", [128, 128], F16, isOutput=False)
    out_d = nc.declare_dram_parameter("out", [STEPS, HO, WO], F16, isOutput=True)

    with tile.TileContext(nc) as tc, ExitStack() as ctx:
        const = ctx.enter_context(tc.tile_pool(name="const", bufs=1))
        fpool = ctx.enter_context(tc.tile_pool(name="fpool", bufs=1))
        dpool = ctx.enter_context(tc.tile_pool(name="dpool", bufs=1))
        upool = ctx.enter_context(tc.tile_pool(name="upool", bufs=2))
        vpool = ctx.enter_context(tc.tile_pool(name="vpool", bufs=2))
        tpool = ctx.enter_context(tc.tile_pool(name="tpool", bufs=1))
        rpool = ctx.enter_context(tc.tile_pool(name="rpool", bufs=1))
        opool = ctx.enter_context(tc.tile_pool(name="opool", bufs=2))
        ps_conv = ctx.enter_context(tc.tile_pool(name="ps_conv", bufs=2, space="PSUM"))
        ps_head = ctx.enter_context(tc.tile_pool(name="ps_head", bufs=1, space="PSUM"))

        # ---- constants ----
        ut_sb = const.tile([64, WO], F16, tag="ut")
        nc.sync.dma_start(out=ut_sb[:], in_=ut_d[:])
        eye_sb = const.tile([128, 128], F16, tag="eye")
        nc.sync.dma_start(out=eye_sb[:], in_=eye_d[:])
        invw_sb, absw_sb, signw_sb = [], [], []
        for ct in range(NCT):
            t = const.tile([128, 1], F32, tag=f"invw{ct}")
            nc.sync.dma_start(out=t[:], in_=invw_d[ct])
            invw_sb.append(t)
            t = const.tile([128, 1], F32, tag=f"absw{ct}")
            nc.sync.dma_start(out=t[:], in_=absw_d[ct])
            absw_sb.append(t)
            t = const.tile([128, 1], F16, tag=f"signw{ct}")
            nc.sync.dma_start(out=t[:], in_=signw_d[ct])
            signw_sb.append(t)

        # ---- fp16 flat variants ----
        fsb = []
        for ct in range(NCT):
            t = fpool.tile([128, FTOT], F16, tag=f"f{ct}")
            nc.sync.dma_start(out=t[:], in_=fblk_d[ct])
            fsb.append(t)

        for s in range(STEPS):
            # stationaries for this step
            dg = dpool.tile([128, NCT * 3 * NPT * 128], F16, tag="diag")
            nc.sync.dma_start(out=dg[:], in_=diag_d[s])
            dgv = dg[:].rearrange("p (ct c pt x) -> p ct c pt x", ct=NCT, c=3, pt=NPT)
            wy = dpool.tile([128, NCT * 3 * 2], F32, tag="wdy")
            nc.sync.dma_start(out=wy[:], in_=wdy_d[s])

            redw = rpool.tile([128, NCT * HW], F16, tag="redw")
            resp_ct, e_ct, tf_ct = [], [], []
            # conv for both ct halves first
            for ct in range(NCT):
                resp = []
                for cv in range(3):
                    rsp = vpool.tile([128, HW], F16, tag=f"rsp{cv}")
                    resp.append(rsp)
                    tmp0 = tpool.tile([128, HW], F16, tag="tmp0")
                    tmp1 = tpool.tile([128, HW], F16, tag="tmp1")
                    wix = (ct * 3 + cv) * 2
                    nc.vector.tensor_scalar(
                        tmp0[:], fsb[ct][:, DYTAP_OFFS[0]: DYTAP_OFFS[0] + HW],
                        wy[:, wix: wix + 1], None, op0=ALU.mult)
                    nc.vector.tensor_scalar(
                        tmp1[:], fsb[ct][:, DYTAP_OFFS[1]: DYTAP_OFFS[1] + HW],
                        wy[:, wix + 1: wix + 2], None, op0=ALU.mult)
                    for g in range(NGR):
                        acc = ps_conv.tile([128, GR], F32, tag="acc")
                        for pi in range(NPT):
                            stat = dgv[:, ct, cv, pi, :]
                            off = PTAP_OFFS[pi] + g * GR
                            for sub in range(GR // 512):
                                nc.tensor.matmul(
                                    acc[:, sub * 512: (sub + 1) * 512],
                                    stat,
                                    fsb[ct][:, off + sub * 512: off + (sub + 1) * 512],
                                    start=(pi == 0),
                                    stop=(pi == NPT - 1),
                                )
                        gsl = slice(g * GR, (g + 1) * GR)
                        nc.vector.tensor_add(rsp[:, gsl], acc[:], tmp0[:, gsl])
                    nc.vector.tensor_add(rsp[:], rsp[:], tmp1[:])
                resp_ct.append(resp)

            # exps batched (one ACT table)
            for ct in range(NCT):
                e0 = upool.tile([128, HW], F16, tag="e0")
                e1 = upool.tile([128, HW], F16, tag="e1")
                e_ct.append((e0, e1))
                nc.scalar.activation(e0[:], resp_ct[ct][0][:], AF.Exp)
                nc.scalar.activation(e1[:], resp_ct[ct][1][:], AF.Exp)
            # t-mults and u; den on GPSIMD
            for ct in range(NCT):
                D0r, D1r, R2r = resp_ct[ct]
                e0, e1 = e_ct[ct]
                t1f = upool.tile([128, HW], F16, tag="t1f")
                tf_ct.append(t1f)
                nc.vector.tensor_mul(D0r[:], D0r[:], e0[:])
                nc.vector.tensor_mul(t1f[:], D1r[:], e1[:])
                nc.vector.tensor_add(t1f[:], D0r[:], t1f[:])
                nc.gpsimd.tensor_add(e0[:], e0[:], e1[:])
            # ln batched, then rw-exp batched
            for ct in range(NCT):
                e0, e1 = e_ct[ct]
                nc.scalar.activation(e1[:], e0[:], AF.Ln, scale=invw_sb[ct][:, 0:1], bias=invw_sb[ct][:, 0:1])
            for ct in range(NCT):
                e0, e1 = e_ct[ct]
                nc.scalar.activation(e1[:], e1[:], AF.Exp, scale=-1.0)
            # v, r2w, redw
            for ct in range(NCT):
                D0r, D1r, R2r = resp_ct[ct]
                e0, e1 = e_ct[ct]
                t1f = tf_ct[ct]
                nc.vector.tensor_mul(t1f[:], t1f[:], e1[:])
                nc.vector.tensor_scalar(D1r[:], R2r[:], absw_sb[ct][:, 0:1], None, op0=ALU.mult)
                nc.gpsimd.tensor_add(redw[:, ct * HW: (ct + 1) * HW], t1f[:], D1r[:])

            # ---- head: dmap[1, pix] = sum_c sign(w)*redw ----
            dmY = opool.tile([64, 64], F16, tag="dmY")
            for k in range(HW // 512):
                pd = ps_head.tile([1, 512], F32, tag="pd")
                for ct in range(NCT):
                    nc.tensor.matmul(
                        pd[:],
                        signw_sb[ct][:],
                        redw[:, ct * HW + k * 512: ct * HW + (k + 1) * 512],
                        start=(ct == 0),
                        stop=(ct == NCT - 1),
                    )
                dm1 = opool.tile([1, 512], F16, tag="dm1")
                nc.vector.tensor_scalar(dm1[:], pd[:], b_val, 0.0, op0=ALU.add, op1=ALU.max)
                nc.sync.dma_start(
                    out=dmY[8 * k: 8 * k + 8, :],
                    in_=dm1[:].rearrange("p (y x) -> p y x", x=64),
                )
            # transpose -> x on partitions
            psT0 = ps_head.tile([64, 64], F16, tag="ups")
            nc.tensor.transpose(psT0[:], dmY[:], eye_sb[0:64, 0:64])
            dmX = opool.tile([64, 64], F16, tag="dmX")
            nc.vector.tensor_copy(dmX[:], psT0[:])

            # horizontal upsample
            ps_h = ps_head.tile([128, 512], F32, tag="ups")
            for xc in range(4):
                nc.tensor.matmul(
                    ps_h[:, xc * 64: (xc + 1) * 64],
                    ut_sb[:, xc * 128: (xc + 1) * 128],
                    dmX[:],
                    start=True,
                    stop=True,
                )
            h_sb = opool.tile([128, 256], F16, tag="h_sb")
            nc.scalar.activation(h_sb[:], ps_h[:, 0:256], AF.Copy)
            hyT = opool.tile([64, 512], F16, tag="hyT")
            for xc in range(4):
                psTx = ps_head.tile([64, 128], F16, tag="ups")
                nc.tensor.transpose(psTx[:], h_sb[:, xc * 64: (xc + 1) * 64], eye_sb[:])
                nc.vector.tensor_copy(hyT[:, xc * 128: (xc + 1) * 128], psTx[:])
            # vertical upsample
            for yc in range(4):
                pv = ps_head.tile([128, 512], F32, tag="ups")
                nc.tensor.matmul(
                    pv[:],
                    ut_sb[:, yc * 128: (yc + 1) * 128],
                    hyT[:],
                    start=True,
                    stop=True,
                )
                osb = opool.tile([128, 512], F16, tag="osb")
                nc.vector.tensor_copy(osb[:], pv[:])
                nc.sync.dma_start(out=out_d[s, yc * 128: (yc + 1) * 128, :], in_=osb[:])

    nc.compile()
    return nc


_CACHE = {}


def _get_nc(b_val: float) -> bass.Bass:
    key = round(b_val, 12)
    if key not in _CACHE:
        _CACHE[key] = _build_nc(b_val)
    return _CACHE[key]


def kernel(f_e, all_prototypes, w_head, b_head):
    in_maps, b_val = _host_prep(f_e, all_prototypes, w_head, b_head)
    nc = _get_nc(b_val)
    res = run_bass_kernel_spmd(nc, in_maps, list(range(NCORES)), trace=False)
    outs = [res.results[b]["out"].reshape(STEPS, 1, HO, WO) for b in range(BS)]
    full = np.stack(outs, axis=1)  # [STEPS, BS, 1, HO, WO]
    return full.astype(np.float32)
